# revision 48
# baseline (speedup 1.0000x reference)
"""Trainium2 Bass kernel for nn_DifferentiableHMM_Centered.

Contract: kernel(**inputs) takes FULL unsharded inputs (same keys as
reference.setup_inputs()) and returns the FULL output (norm_copy [S,B] f32,
smooth_loss scalar f32), distributing across 8 NeuronCores internally.

Math reduction
--------------
The reference's straight-through output is, up to ~1ulp wiggle,
  norm_copy[s,b] = all_means[k*(s,b)],
  smooth_loss    = 0.1 * 2 * #{(e,b): k*(row_e,b) != k*(col_e,b)} / (E*Bc*3)
where k*(s,b) = argmax_k [ -0.5 z_k^2 - log(std_k) + gumbel[s,b,k] ],
z_k = (x[s,b]-mean_k)/std_k  (softmax/TAU/one-hot are monotone decorations).

Relative to state 0 the decision statistic is quadratic in x:
  D_k := d_k - d_0 = -(A_k*x^2 + B_k*x) + (g_k - g_0) - const_k
so the device only needs x and the host-precomputed G_k = (g_k-g_0) - const_k.
k* = 0 if D_1<=0 and D_2<=0; 1 if D_1>0 and D_2<=D_1; else 2
(strict comparisons reproduce jnp.argmax first-index tie-breaking).

Device plan (8 cores, SPMD, one NEFF):
  phase 1 (spots sharded 256/core): D_k via fused scalar_tensor_tensor on DVE
    (all per-state scalars baked as immediates), masks t01/t2, codes {0,1,2}
    + norm_copy (ACT Identity affine of the code); codes cast f32->int8 on
    store. All elementwise runs on DVE/ACT -- GpSimd compute contends with
    DVE on SBUF ports (measured 15x slowdown), so GpSimd only drives DMA.
  AllGather of int8 codes (rows padded to 3072B), split per 128-spot tile so
    the first AG overlaps the second tile's compute.
  phase 2 (edges sharded 1536/core): dma_gather of row/col code rows,
    fused scalar_tensor_tensor(bypass, not_equal, accum_out) -> counts.
Host sums the 8x128xNCH partial counts into smooth_loss.
"""

import numpy as np

S, B, E = 2048, 3000, 12288
M = 8            # cores
SS = S // M      # 256 spots per core
CP = 3072        # padded code row length (int8 bytes), multiple of 256
CHUNKS = (896, 768)  # phase-2 chunk sizes (multiples of 128)
ECAP = sum(CHUNKS)   # per-core edge capacity (edges partitioned by row owner,
                     # padded with self-comparing slots; E/M=1536, max seen 1576)
NCH = len(CHUNKS)
TAU = 0.1
LAMBDA_SMOOTH = 0.1

_prog_cache: dict = {}
_TRACE = False       # set by test harness for profiling runs
_last_result = None  # BassKernelResults of the most recent launch
_last_results = []   # all launches of the most recent kernel() call
MODE = "two"         # "two" = phase1 NEFF + host relay + phase2 NEFF
                     # "one" = single NEFF with on-device AllGather


def _build_program(cfg, do_p1=True, do_ag=True, do_p2=True, split_ag=True):
    """cfg: (a_zero, norm_linear, A1, A2, B1, B2, beta, gamma, sm0, sm1).
    All scalars are baked into the BIR as immediates."""
    import concourse.bass as bass
    import concourse.bacc as bacc
    import concourse.tile as tile
    from concourse import mybir

    a_zero, norm_linear, A1, A2, B1, B2, beta, gamma, sm0, sm1 = cfg

    dt = mybir.dt
    f32 = dt.float32
    Alu = mybir.AluOpType
    Act = mybir.ActivationFunctionType
    ts = bass.ts

    nc = bacc.Bacc(
        "TRN2",
        target_bir_lowering=False,
        debug=False,
        num_devices=M,
    )

    x_in = nc.dram_tensor("x_sh", [SS, B], f32, kind="ExternalInput").ap()
    g1_in = nc.dram_tensor("g1_sh", [SS, B], f32, kind="ExternalInput").ap()
    g2_in = nc.dram_tensor("g2_sh", [SS, B], f32, kind="ExternalInput").ap()
    ridx_in = nc.dram_tensor("ridx", [128, ECAP // 16], dt.int16, kind="ExternalInput").ap()
    cidx_in = nc.dram_tensor("cidx", [128, ECAP // 16], dt.int16, kind="ExternalInput").ap()
    norm_out = nc.dram_tensor("norm_out", [SS, B], f32, kind="ExternalOutput").ap()
    cnt_out = nc.dram_tensor("cnt_out", [128, NCH], f32, kind="ExternalOutput").ap()

    NT = SS // 128  # spot tiles per core (2)

    with tile.TileContext(nc) as tc:
        with tc.tile_pool(name="dram", bufs=1, space="DRAM") as dpool, \
             tc.tile_pool(name="consts", bufs=1) as cpool:

            codes_shard = dpool.tile([SS, CP], dt.int8)
            codes_full = dpool.tile([S, CP], dt.int8, addr_space="Shared")

            # constant data tile for copy_predicated (code 2 where state-2 wins)
            two_t = cpool.tile([128, B], f32)
            nc.vector.memset(two_t[:], 2.0)
            gamma_t = cpool.tile([128, 1], f32)
            nc.vector.memset(gamma_t[:], float(gamma))
            if not norm_linear:
                sm1_t = cpool.tile([128, B], f32)
                nc.vector.memset(sm1_t[:], float(sm1))

            ridx = cpool.tile([128, ECAP // 16], dt.int16)
            nc.sync.dma_start(ridx[:], ridx_in[:])
            cidx = cpool.tile([128, ECAP // 16], dt.int16)
            nc.sync.dma_start(cidx[:], cidx_in[:])

            with tc.tile_pool(name="p1", bufs=2) as p1:
                for t in range(NT if do_p1 else 0):
                    rows = ts(t, 128)
                    xs = p1.tile([128, B], f32, tag="xs")
                    nc.sync.dma_start(xs[:], x_in[rows, :])
                    g1t = p1.tile([128, B], f32, tag="g1")
                    nc.sync.dma_start(g1t[:], g1_in[rows, :])
                    g2t = p1.tile([128, B], f32, tag="g2")
                    nc.sync.dma_start(g2t[:], g2_in[rows, :])

                    # D_k computed in place over G_k tiles (DVE)
                    if a_zero:
                        nc.vector.scalar_tensor_tensor(
                            g1t[:], xs[:], -B1, g1t[:],
                            op0=Alu.mult, op1=Alu.add)
                        nc.vector.scalar_tensor_tensor(
                            g2t[:], xs[:], -B2, g2t[:],
                            op0=Alu.mult, op1=Alu.add)
                    else:
                        y = p1.tile([128, B], f32, tag="y")
                        nc.scalar.activation(y[:], xs[:], Act.Square)
                        nc.vector.scalar_tensor_tensor(
                            g1t[:], y[:], -A1, g1t[:],
                            op0=Alu.mult, op1=Alu.add)
                        nc.vector.scalar_tensor_tensor(
                            g1t[:], xs[:], -B1, g1t[:],
                            op0=Alu.mult, op1=Alu.add)
                        nc.vector.scalar_tensor_tensor(
                            g2t[:], y[:], -A2, g2t[:],
                            op0=Alu.mult, op1=Alu.add)
                        nc.vector.scalar_tensor_tensor(
                            g2t[:], xs[:], -B2, g2t[:],
                            op0=Alu.mult, op1=Alu.add)

                    code = p1.tile([128, CP], f32, tag="code")
                    t01 = code[:, 0:B]
                    # t01 = [D1 > 0]  (DVE tensor_scalar, 2x mode)
                    nc.vector.tensor_scalar(t01, g1t[:], 0.0, None, op0=Alu.is_gt)
                    # mA = max(D1, 0) = relu(D1)  (ACT)
                    mA = p1.tile([128, B], f32, tag="mA")
                    nc.scalar.activation(mA[:], g1t[:], Act.Relu)
                    # t2 = [D2 > mA], in place over D2, uint32 mask for
                    # copy_predicated (integer mask required)
                    t2 = g2t[:].bitcast(dt.uint32)
                    nc.vector.tensor_tensor(t2, g2t[:], mA[:], op=Alu.is_gt)
                    nc.vector.copy_predicated(t01, t2, two_t[:])
                    # pad cols [B:CP] stay uninitialized -- phase 2 compares
                    # only [:, :, 0:B] of each gathered row

                    normt = p1.tile([128, B], f32, tag="normt")
                    if norm_linear:
                        # norm = beta*code + gamma  (ACT Identity affine)
                        nc.scalar.activation(
                            normt[:], t01, Act.Identity,
                            bias=gamma_t[:, 0:1], scale=float(beta))
                    else:
                        nc.vector.tensor_scalar(
                            normt[:], t01, -sm0, sm0, op0=Alu.mult, op1=Alu.add)
                        nc.vector.copy_predicated(normt[:], t2, sm1_t[:])
                    nc.sync.dma_start(norm_out[rows, :], normt[:])
                    # f32 -> int8 cast during store (SWDGE)
                    nc.gpsimd.dma_start(codes_shard[rows, :], code[:])

            if do_ag:
                nc.gpsimd.collective_compute(
                    "AllGather",
                    Alu.bypass,
                    replica_groups=[list(range(M))],
                    ins=[codes_shard[:, :].opt()],
                    outs=[codes_full[:, :].opt()],
                )

            with tc.tile_pool(name="p2", bufs=1) as p2:
                accs = cpool.tile([128, NCH], f32, name="accs")
                nc.vector.memset(accs[:, :], 0.0)
                coff = [sum(CHUNKS[:c]) for c in range(NCH)]
                gathered = []
                # row gathers read the LOCAL codes_shard (edges are
                # partitioned by row owner) -> they overlap the AllGather
                for ch in range(NCH if do_p2 else 0):
                    che = CHUNKS[ch]
                    cols = slice(coff[ch] // 16, (coff[ch] + che) // 16)
                    rt = p2.tile([128, che // 128, CP], dt.int8, tag=f"rt{ch}")
                    nc.gpsimd.dma_gather(
                        rt[:], codes_shard[:, :], ridx[:, cols],
                        num_idxs=che, num_idxs_reg=che, elem_size=CP)
                    gathered.append(rt)
                for ch in range(NCH if do_p2 else 0):
                    che = CHUNKS[ch]
                    cols = slice(coff[ch] // 16, (coff[ch] + che) // 16)
                    ct = p2.tile([128, che // 128, CP], dt.int8, tag=f"ct{ch}")
                    nc.gpsimd.dma_gather(
                        ct[:], codes_full[:, :], cidx[:, cols],
                        num_idxs=che, num_idxs_reg=che, elem_size=CP)
                    rt = gathered[ch]
                    scr = p2.tile([128, che // 128, CP], dt.int8, tag="scr")
                    # compare only the 3000 real code bytes of each row
                    nc.vector.scalar_tensor_tensor(
                        scr[:, :, 0:B],
                        rt[:, :, 0:B],
                        0.0,
                        ct[:, :, 0:B],
                        op0=Alu.bypass,
                        op1=Alu.not_equal,
                        accum_out=accs[:, ch:ch + 1],
                    )
                nc.sync.dma_start(cnt_out[:, :], accs[:, :])

    nc.compile()
    return nc


def _build_phase1(cfg):
    """Phase-1-only NEFF: no collectives -> no NRT comm-init barrier.
    Outputs norm_copy shard + int8 codes shard (padded rows)."""
    import concourse.bacc as bacc
    import concourse.tile as tile
    from concourse import mybir

    a_zero, norm_linear, A1, A2, B1, B2, beta, gamma, sm0, sm1 = cfg
    dt = mybir.dt
    f32 = dt.float32
    Alu = mybir.AluOpType
    Act = mybir.ActivationFunctionType
    import concourse.bass as bass
    ts = bass.ts

    nc = bacc.Bacc("TRN2", target_bir_lowering=False, debug=False, num_devices=M)
    x_in = nc.dram_tensor("x_sh", [SS, B], f32, kind="ExternalInput").ap()
    g1_in = nc.dram_tensor("g1_sh", [SS, B], f32, kind="ExternalInput").ap()
    g2_in = nc.dram_tensor("g2_sh", [SS, B], f32, kind="ExternalInput").ap()
    norm_out = nc.dram_tensor("norm_out", [SS, B], f32, kind="ExternalOutput").ap()
    codes_out = nc.dram_tensor("codes_out", [SS, CP], dt.int8, kind="ExternalOutput").ap()

    with tile.TileContext(nc) as tc:
        with tc.tile_pool(name="consts", bufs=1) as cpool:
            two_t = cpool.tile([128, B], f32)
            nc.vector.memset(two_t[:], 2.0)
            gamma_t = cpool.tile([128, 1], f32)
            nc.vector.memset(gamma_t[:], float(gamma))
            if not norm_linear:
                sm1_t = cpool.tile([128, B], f32)
                nc.vector.memset(sm1_t[:], float(sm1))

            HB = B // 2  # process 1500-col halves so compute starts earlier
            with tc.tile_pool(name="p1", bufs=3) as p1:
                for t in range(SS // 128):
                    rows = ts(t, 128)
                    code = p1.tile([128, CP], f32, tag="code")
                    normt = p1.tile([128, B], f32, tag="normt")
                    for h in range(2):
                        cols = ts(h, HB)
                        xs = p1.tile([128, HB], f32, tag="xs")
                        nc.sync.dma_start(xs[:], x_in[rows, cols])
                        g1t = p1.tile([128, HB], f32, tag="g1")
                        nc.sync.dma_start(g1t[:], g1_in[rows, cols])
                        g2t = p1.tile([128, HB], f32, tag="g2")
                        nc.sync.dma_start(g2t[:], g2_in[rows, cols])

                        if a_zero:
                            nc.vector.scalar_tensor_tensor(
                                g1t[:], xs[:], -B1, g1t[:], op0=Alu.mult, op1=Alu.add)
                            nc.vector.scalar_tensor_tensor(
                                g2t[:], xs[:], -B2, g2t[:], op0=Alu.mult, op1=Alu.add)
                        else:
                            y = p1.tile([128, HB], f32, tag="y")
                            nc.scalar.activation(y[:], xs[:], Act.Square)
                            nc.vector.scalar_tensor_tensor(
                                g1t[:], y[:], -A1, g1t[:], op0=Alu.mult, op1=Alu.add)
                            nc.vector.scalar_tensor_tensor(
                                g1t[:], xs[:], -B1, g1t[:], op0=Alu.mult, op1=Alu.add)
                            nc.vector.scalar_tensor_tensor(
                                g2t[:], y[:], -A2, g2t[:], op0=Alu.mult, op1=Alu.add)
                            nc.vector.scalar_tensor_tensor(
                                g2t[:], xs[:], -B2, g2t[:], op0=Alu.mult, op1=Alu.add)

                        t01 = code[:, cols]
                        nc.vector.tensor_scalar(t01, g1t[:], 0.0, None, op0=Alu.is_gt)
                        mA = p1.tile([128, HB], f32, tag="mA")
                        nc.scalar.activation(mA[:], g1t[:], Act.Relu)
                        t2 = g2t[:].bitcast(dt.uint32)
                        nc.vector.tensor_tensor(t2, g2t[:], mA[:], op=Alu.is_gt)
                        nc.vector.copy_predicated(t01, t2, two_t[:, 0:HB])
                        if norm_linear:
                            nc.scalar.activation(
                                normt[:, cols], t01, Act.Identity,
                                bias=gamma_t[:, 0:1], scale=float(beta))
                        else:
                            nc.vector.tensor_scalar(
                                normt[:, cols], t01, -sm0, sm0,
                                op0=Alu.mult, op1=Alu.add)
                            nc.vector.copy_predicated(
                                normt[:, cols], t2, sm1_t[:, 0:HB])
                        # f32 -> int8 cast during store (SWDGE), per half so
                        # the first store overlaps the second half's compute;
                        # pad cols garbage, phase 2 compares only [0:B]
                        nc.gpsimd.dma_start(codes_out[rows, cols], code[:, cols])
                        nc.sync.dma_start(norm_out[rows, cols], normt[:, cols])
                    nc.gpsimd.dma_start(codes_out[rows, B:CP], code[:, B:CP])

    nc.compile()
    return nc


P2CHUNKS = (256, 640, 640)  # phase-2 chunks: small first chunk starts the
                            # DVE compare ~10us earlier; later chunks' gather
                            # issues (~9ns/row, serial on Q7) hide under the
                            # running compares. (768,768), (256,512,768) and
                            # (384,640,512) all measured slower.


def _build_phase2():
    """Phase-2-only NEFF: codes for all spots arrive as a replicated input;
    gather row/col code rows per edge, count mismatches."""
    import concourse.bacc as bacc
    import concourse.tile as tile
    from concourse import mybir

    dt = mybir.dt
    f32 = dt.float32
    Alu = mybir.AluOpType

    EC = E // M
    NC2 = len(P2CHUNKS)
    nc = bacc.Bacc("TRN2", target_bir_lowering=False, debug=False, num_devices=M)
    codes_in = nc.dram_tensor("codes_full", [S, CP], dt.int8, kind="ExternalInput").ap()
    ridx_in = nc.dram_tensor("ridx", [128, EC // 16], dt.int16, kind="ExternalInput").ap()
    cidx_in = nc.dram_tensor("cidx", [128, EC // 16], dt.int16, kind="ExternalInput").ap()
    cnt_out = nc.dram_tensor("cnt_out", [128, NC2], f32, kind="ExternalOutput").ap()

    with tile.TileContext(nc) as tc:
        with tc.tile_pool(name="c2", bufs=1) as cpool:
            ridx = cpool.tile([128, EC // 16], dt.int16)
            nc.sync.dma_start(ridx[:], ridx_in[:])
            cidx = cpool.tile([128, EC // 16], dt.int16)
            nc.sync.dma_start(cidx[:], cidx_in[:])
            accs = cpool.tile([128, NC2], f32, name="accs")
            nc.vector.memset(accs[:, :], 0.0)

            with tc.tile_pool(name="p2", bufs=1) as p2:
                coff = [sum(P2CHUNKS[:c]) for c in range(NC2)]
                pairs = []
                for ch in range(NC2):
                    che = P2CHUNKS[ch]
                    cols = slice(coff[ch] // 16, (coff[ch] + che) // 16)
                    rt = p2.tile([128, che // 128, CP], dt.int8, tag=f"rt{ch}")
                    nc.gpsimd.dma_gather(
                        rt[:], codes_in[:, :], ridx[:, cols],
                        num_idxs=che, num_idxs_reg=che, elem_size=CP)
                    ct = p2.tile([128, che // 128, CP], dt.int8, tag=f"ct{ch}")
                    nc.gpsimd.dma_gather(
                        ct[:], codes_in[:, :], cidx[:, cols],
                        num_idxs=che, num_idxs_reg=che, elem_size=CP)
                    pairs.append((rt, ct))
                for ch, (rt, ct) in enumerate(pairs):
                    che = P2CHUNKS[ch]
                    scr = p2.tile([128, che // 128, CP], dt.int8, tag="scr")
                    nc.vector.scalar_tensor_tensor(
                        scr[:, :, 0:B],
                        rt[:, :, 0:B],
                        0.0,
                        ct[:, :, 0:B],
                        op0=Alu.bypass,
                        op1=Alu.not_equal,
                        accum_out=accs[:, ch:ch + 1],
                    )
                nc.sync.dma_start(cnt_out[:, :], accs[:, :])

    nc.compile()
    return nc


def _host_prep(x, bin_idx, edge_index, gumbel_noise, state_means, log_stds):
    """Shard + precompute per-core input maps and the baked-scalar config."""
    x = np.asarray(x, dtype=np.float32)
    bin_idx = np.asarray(bin_idx)
    edge_index = np.asarray(edge_index)
    gn = np.asarray(gumbel_noise, dtype=np.float32)
    sm = np.asarray(state_means, dtype=np.float32)
    ls = np.asarray(log_stds, dtype=np.float32)

    if not np.array_equal(bin_idx, np.arange(x.shape[1], dtype=bin_idx.dtype)):
        x = np.ascontiguousarray(x[:, bin_idx])
        gn = np.ascontiguousarray(gn[:, bin_idx, :])

    stds = (np.exp(ls.astype(np.float64)) + 1e-6)
    means = np.array([float(sm[0]), 0.0, float(sm[1])], dtype=np.float64)
    inv_var = 1.0 / (stds * stds)
    # D_k = -(A_k x^2 + B_k x) + (g_k - g_0) - (C_k + log(std_k/std_0))
    A = 0.5 * (inv_var - inv_var[0])
    Bc_ = -(means * inv_var - means[0] * inv_var[0])
    Cc = 0.5 * (means * means * inv_var - means[0] * means[0] * inv_var[0]) \
        + np.log(stds) - np.log(stds[0])

    A1, A2 = np.float32(A[1]), np.float32(A[2])
    B1, B2 = np.float32(Bc_[1]), np.float32(Bc_[2])
    C1, C2 = np.float32(Cc[1]), np.float32(Cc[2])
    a_zero = (A1 == 0.0) and (A2 == 0.0)

    # norm_copy = alpha*code^2 + beta*code + gamma; linear iff alpha == 0
    alpha = (means[0] + means[2]) / 2.0
    beta = -(3.0 * means[0] + means[2]) / 2.0
    gamma = means[0]
    norm_linear = np.float32(alpha) == 0.0

    cfg = (bool(a_zero), bool(norm_linear), float(A1), float(A2),
           float(B1), float(B2), float(np.float32(beta)),
           float(np.float32(gamma)), float(means[0]), float(means[2]))

    G1 = (gn[:, :, 1] - gn[:, :, 0]) - C1
    G2 = (gn[:, :, 2] - gn[:, :, 0]) - C2

    def wrap_idx(idx):
        # element i -> [i % 16, i // 16], replicated 8x down the 128
        # partitions (each GPSIMD Q7 core reads its own 16-partition copy)
        n = idx.shape[0]
        w = np.zeros((16, n // 16), dtype=np.int16)
        w[np.arange(n) % 16, np.arange(n) // 16] = idx.astype(np.int16)
        return np.ascontiguousarray(np.tile(w, (8, 1)))

    # partition edges by row owner; row gathers then read the local shard
    # (overlapping the AllGather), col gathers read the allgathered rows
    # (single AllGather concatenates rank shards in order: row == spot id)
    erow = edge_index[0].astype(np.int64)
    ecol = edge_index[1].astype(np.int64)
    owner = erow // SS

    EC = E // M
    in_maps = []      # single-launch (mode "one"): by-owner edges, padded
    p2_maps = []      # two-launch (mode "two"): plain E/M split, global rows
    for i in range(M):
        r0, r1 = SS * i, SS * (i + 1)
        sel = owner == i
        er = erow[sel] - r0          # local row index into codes_shard
        ec = ecol[sel]               # global row index into codes_full
        n = er.shape[0]
        entry = {
            "x_sh": np.ascontiguousarray(x[r0:r1]),
            "g1_sh": np.ascontiguousarray(G1[r0:r1]),
            "g2_sh": np.ascontiguousarray(G2[r0:r1]),
        }
        if n <= ECAP:
            # pad with self-comparing slots: local row 0 vs its global id
            er_p = np.full(ECAP, 0, dtype=np.int64)
            ec_p = np.full(ECAP, r0, dtype=np.int64)
            er_p[:n] = er
            ec_p[:n] = ec
            entry["ridx"] = wrap_idx(er_p)
            entry["cidx"] = wrap_idx(ec_p)
        in_maps.append(entry)
        p2_maps.append({
            "ridx": wrap_idx(erow[EC * i:EC * (i + 1)]),
            "cidx": wrap_idx(ecol[EC * i:EC * (i + 1)]),
        })
    return in_maps, p2_maps, cfg


def kernel(x, bin_idx, edge_index, gumbel_noise, state_means, log_stds):
    from concourse.bass_utils import run_bass_kernel_spmd

    in_maps, p2_maps, cfg = _host_prep(
        x, bin_idx, edge_index, gumbel_noise, state_means, log_stds)

    global _last_result, _last_results
    _last_results = []
    cores = list(range(M))

    if MODE == "one":
        key = ("one", cfg)
        if key not in _prog_cache:
            _prog_cache[key] = _build_program(cfg)
        res = run_bass_kernel_spmd(_prog_cache[key], in_maps, core_ids=cores,
                                   trace=_TRACE)
        _last_result = res
        _last_results = [res]
        outs = res.results
    else:
        key1 = ("p1", cfg)
        if key1 not in _prog_cache:
            _prog_cache[key1] = _build_phase1(cfg)
        if "p2" not in _prog_cache:
            _prog_cache["p2"] = _build_phase2()
        maps1 = [{k: m[k] for k in ("x_sh", "g1_sh", "g2_sh")} for m in in_maps]
        res1 = run_bass_kernel_spmd(_prog_cache[key1], maps1, core_ids=cores,
                                    trace=_TRACE)
        codes_full = np.concatenate(
            [res1.results[i]["codes_out"] for i in range(M)], axis=0)
        maps2 = [{"codes_full": codes_full, **p2_maps[i]} for i in range(M)]
        res2 = run_bass_kernel_spmd(_prog_cache["p2"], maps2, core_ids=cores,
                                    trace=_TRACE)
        _last_result = res2
        _last_results = [res1, res2]
        outs = [{**res1.results[i], **res2.results[i]} for i in range(M)]

    norm_copy = np.concatenate([outs[i]["norm_out"] for i in range(M)], axis=0)
    total = np.float64(0.0)
    for i in range(M):
        total += np.float64(outs[i]["cnt_out"].sum(dtype=np.float64))
    loss = np.float32(LAMBDA_SMOOTH * 2.0 * total / (E * B * 3))
    return norm_copy.astype(np.float32), loss


# revision 49
# speedup vs baseline: 1.0592x; 1.0592x over previous
"""Trainium2 Bass kernel for nn_DifferentiableHMM_Centered.

Contract: kernel(**inputs) takes FULL unsharded inputs (same keys as
reference.setup_inputs()) and returns the FULL output (norm_copy [S,B] f32,
smooth_loss scalar f32), distributing across 8 NeuronCores internally.

Math reduction
--------------
The reference's straight-through output is, up to ~1ulp wiggle,
  norm_copy[s,b] = all_means[k*(s,b)],
  smooth_loss    = 0.1 * 2 * #{(e,b): k*(row_e,b) != k*(col_e,b)} / (E*Bc*3)
where k*(s,b) = argmax_k [ -0.5 z_k^2 - log(std_k) + gumbel[s,b,k] ],
z_k = (x[s,b]-mean_k)/std_k  (softmax/TAU/one-hot are monotone decorations).

Relative to state 0 the decision statistic is quadratic in x:
  D_k := d_k - d_0 = -(A_k*x^2 + B_k*x) + (g_k - g_0) - const_k
so the device only needs x and the host-precomputed G_k = (g_k-g_0) - const_k.
k* = 0 if D_1<=0 and D_2<=0; 1 if D_1>0 and D_2<=D_1; else 2
(strict comparisons reproduce jnp.argmax first-index tie-breaking).

Device plan (8 cores, SPMD, one NEFF):
  phase 1 (spots sharded 256/core): D_k via fused scalar_tensor_tensor on DVE
    (all per-state scalars baked as immediates), masks t01/t2, codes {0,1,2}
    + norm_copy (ACT Identity affine of the code); codes cast f32->int8 on
    store. All elementwise runs on DVE/ACT -- GpSimd compute contends with
    DVE on SBUF ports (measured 15x slowdown), so GpSimd only drives DMA.
  AllGather of int8 codes (rows padded to 3072B), split per 128-spot tile so
    the first AG overlaps the second tile's compute.
  phase 2 (edges sharded 1536/core): dma_gather of row/col code rows,
    fused scalar_tensor_tensor(bypass, not_equal, accum_out) -> counts.
Host sums the 8x128xNCH partial counts into smooth_loss.
"""

import numpy as np

S, B, E = 2048, 3000, 12288
M = 8            # cores
SS = S // M      # 256 spots per core
CP = 3072        # padded code row length (int8 bytes), multiple of 256
CHUNKS = (896, 768)  # phase-2 chunk sizes (multiples of 128)
ECAP = sum(CHUNKS)   # per-core edge capacity (edges partitioned by row owner,
                     # padded with self-comparing slots; E/M=1536, max seen 1576)
NCH = len(CHUNKS)
TAU = 0.1
LAMBDA_SMOOTH = 0.1

_prog_cache: dict = {}
_TRACE = False       # set by test harness for profiling runs
_last_result = None  # BassKernelResults of the most recent launch
_last_results = []   # all launches of the most recent kernel() call
MODE = "two"         # "two" = phase1 NEFF + host relay + phase2 NEFF
                     # "one" = single NEFF with on-device AllGather


def _build_program(cfg, do_p1=True, do_ag=True, do_p2=True, split_ag=True):
    """cfg: (a_zero, norm_linear, A1, A2, B1, B2, beta, gamma, sm0, sm1).
    All scalars are baked into the BIR as immediates."""
    import concourse.bass as bass
    import concourse.bacc as bacc
    import concourse.tile as tile
    from concourse import mybir

    a_zero, norm_linear, A1, A2, B1, B2, beta, gamma, sm0, sm1 = cfg

    dt = mybir.dt
    f32 = dt.float32
    Alu = mybir.AluOpType
    Act = mybir.ActivationFunctionType
    ts = bass.ts

    nc = bacc.Bacc(
        "TRN2",
        target_bir_lowering=False,
        debug=False,
        num_devices=M,
    )

    x_in = nc.dram_tensor("x_sh", [SS, B], f32, kind="ExternalInput").ap()
    g1_in = nc.dram_tensor("g1_sh", [SS, B], f32, kind="ExternalInput").ap()
    g2_in = nc.dram_tensor("g2_sh", [SS, B], f32, kind="ExternalInput").ap()
    ridx_in = nc.dram_tensor("ridx", [128, ECAP // 16], dt.int16, kind="ExternalInput").ap()
    cidx_in = nc.dram_tensor("cidx", [128, ECAP // 16], dt.int16, kind="ExternalInput").ap()
    norm_out = nc.dram_tensor("norm_out", [SS, B], f32, kind="ExternalOutput").ap()
    cnt_out = nc.dram_tensor("cnt_out", [128, NCH], f32, kind="ExternalOutput").ap()

    NT = SS // 128  # spot tiles per core (2)

    with tile.TileContext(nc) as tc:
        with tc.tile_pool(name="dram", bufs=1, space="DRAM") as dpool, \
             tc.tile_pool(name="consts", bufs=1) as cpool:

            codes_shard = dpool.tile([SS, CP], dt.int8)
            codes_full = dpool.tile([S, CP], dt.int8, addr_space="Shared")

            # constant data tile for copy_predicated (code 2 where state-2 wins)
            two_t = cpool.tile([128, B], f32)
            nc.vector.memset(two_t[:], 2.0)
            gamma_t = cpool.tile([128, 1], f32)
            nc.vector.memset(gamma_t[:], float(gamma))
            if not norm_linear:
                sm1_t = cpool.tile([128, B], f32)
                nc.vector.memset(sm1_t[:], float(sm1))

            ridx = cpool.tile([128, ECAP // 16], dt.int16)
            nc.sync.dma_start(ridx[:], ridx_in[:])
            cidx = cpool.tile([128, ECAP // 16], dt.int16)
            nc.sync.dma_start(cidx[:], cidx_in[:])

            with tc.tile_pool(name="p1", bufs=2) as p1:
                for t in range(NT if do_p1 else 0):
                    rows = ts(t, 128)
                    xs = p1.tile([128, B], f32, tag="xs")
                    nc.sync.dma_start(xs[:], x_in[rows, :])
                    g1t = p1.tile([128, B], f32, tag="g1")
                    nc.sync.dma_start(g1t[:], g1_in[rows, :])
                    g2t = p1.tile([128, B], f32, tag="g2")
                    nc.sync.dma_start(g2t[:], g2_in[rows, :])

                    # D_k computed in place over G_k tiles (DVE)
                    if a_zero:
                        nc.vector.scalar_tensor_tensor(
                            g1t[:], xs[:], -B1, g1t[:],
                            op0=Alu.mult, op1=Alu.add)
                        nc.vector.scalar_tensor_tensor(
                            g2t[:], xs[:], -B2, g2t[:],
                            op0=Alu.mult, op1=Alu.add)
                    else:
                        y = p1.tile([128, B], f32, tag="y")
                        nc.scalar.activation(y[:], xs[:], Act.Square)
                        nc.vector.scalar_tensor_tensor(
                            g1t[:], y[:], -A1, g1t[:],
                            op0=Alu.mult, op1=Alu.add)
                        nc.vector.scalar_tensor_tensor(
                            g1t[:], xs[:], -B1, g1t[:],
                            op0=Alu.mult, op1=Alu.add)
                        nc.vector.scalar_tensor_tensor(
                            g2t[:], y[:], -A2, g2t[:],
                            op0=Alu.mult, op1=Alu.add)
                        nc.vector.scalar_tensor_tensor(
                            g2t[:], xs[:], -B2, g2t[:],
                            op0=Alu.mult, op1=Alu.add)

                    code = p1.tile([128, CP], f32, tag="code")
                    t01 = code[:, 0:B]
                    # t01 = [D1 > 0]  (DVE tensor_scalar, 2x mode)
                    nc.vector.tensor_scalar(t01, g1t[:], 0.0, None, op0=Alu.is_gt)
                    # mA = max(D1, 0) = relu(D1)  (ACT)
                    mA = p1.tile([128, B], f32, tag="mA")
                    nc.scalar.activation(mA[:], g1t[:], Act.Relu)
                    # t2 = [D2 > mA], in place over D2, uint32 mask for
                    # copy_predicated (integer mask required)
                    t2 = g2t[:].bitcast(dt.uint32)
                    nc.vector.tensor_tensor(t2, g2t[:], mA[:], op=Alu.is_gt)
                    nc.vector.copy_predicated(t01, t2, two_t[:])
                    # pad cols [B:CP] stay uninitialized -- phase 2 compares
                    # only [:, :, 0:B] of each gathered row

                    normt = p1.tile([128, B], f32, tag="normt")
                    if norm_linear:
                        # norm = beta*code + gamma  (ACT Identity affine)
                        nc.scalar.activation(
                            normt[:], t01, Act.Identity,
                            bias=gamma_t[:, 0:1], scale=float(beta))
                    else:
                        nc.vector.tensor_scalar(
                            normt[:], t01, -sm0, sm0, op0=Alu.mult, op1=Alu.add)
                        nc.vector.copy_predicated(normt[:], t2, sm1_t[:])
                    nc.sync.dma_start(norm_out[rows, :], normt[:])
                    # f32 -> int8 cast during store (SWDGE)
                    nc.gpsimd.dma_start(codes_shard[rows, :], code[:])

            if do_ag:
                nc.gpsimd.collective_compute(
                    "AllGather",
                    Alu.bypass,
                    replica_groups=[list(range(M))],
                    ins=[codes_shard[:, :].opt()],
                    outs=[codes_full[:, :].opt()],
                )

            with tc.tile_pool(name="p2", bufs=1) as p2:
                accs = cpool.tile([128, NCH], f32, name="accs")
                nc.vector.memset(accs[:, :], 0.0)
                coff = [sum(CHUNKS[:c]) for c in range(NCH)]
                gathered = []
                # row gathers read the LOCAL codes_shard (edges are
                # partitioned by row owner) -> they overlap the AllGather
                for ch in range(NCH if do_p2 else 0):
                    che = CHUNKS[ch]
                    cols = slice(coff[ch] // 16, (coff[ch] + che) // 16)
                    rt = p2.tile([128, che // 128, CP], dt.int8, tag=f"rt{ch}")
                    nc.gpsimd.dma_gather(
                        rt[:], codes_shard[:, :], ridx[:, cols],
                        num_idxs=che, num_idxs_reg=che, elem_size=CP)
                    gathered.append(rt)
                for ch in range(NCH if do_p2 else 0):
                    che = CHUNKS[ch]
                    cols = slice(coff[ch] // 16, (coff[ch] + che) // 16)
                    ct = p2.tile([128, che // 128, CP], dt.int8, tag=f"ct{ch}")
                    nc.gpsimd.dma_gather(
                        ct[:], codes_full[:, :], cidx[:, cols],
                        num_idxs=che, num_idxs_reg=che, elem_size=CP)
                    rt = gathered[ch]
                    scr = p2.tile([128, che // 128, CP], dt.int8, tag="scr")
                    # compare only the 3000 real code bytes of each row
                    nc.vector.scalar_tensor_tensor(
                        scr[:, :, 0:B],
                        rt[:, :, 0:B],
                        0.0,
                        ct[:, :, 0:B],
                        op0=Alu.bypass,
                        op1=Alu.not_equal,
                        accum_out=accs[:, ch:ch + 1],
                    )
                nc.sync.dma_start(cnt_out[:, :], accs[:, :])

    nc.compile()
    return nc


def _build_phase1(cfg):
    """Phase-1-only NEFF: no collectives -> no NRT comm-init barrier.
    Outputs norm_copy shard + int8 codes shard (padded rows)."""
    import concourse.bacc as bacc
    import concourse.tile as tile
    from concourse import mybir

    a_zero, norm_linear, A1, A2, B1, B2, beta, gamma, sm0, sm1 = cfg
    dt = mybir.dt
    f32 = dt.float32
    Alu = mybir.AluOpType
    Act = mybir.ActivationFunctionType
    import concourse.bass as bass
    ts = bass.ts

    nc = bacc.Bacc("TRN2", target_bir_lowering=False, debug=False, num_devices=M)
    x_in = nc.dram_tensor("x_sh", [SS, B], f32, kind="ExternalInput").ap()
    g1_in = nc.dram_tensor("g1_sh", [SS, B], f32, kind="ExternalInput").ap()
    g2_in = nc.dram_tensor("g2_sh", [SS, B], f32, kind="ExternalInput").ap()
    norm_out = nc.dram_tensor("norm_out", [SS, B], f32, kind="ExternalOutput").ap()
    codes_out = nc.dram_tensor("codes_out", [SS, CP], dt.int8, kind="ExternalOutput").ap()

    with tile.TileContext(nc) as tc:
        with tc.tile_pool(name="consts", bufs=1) as cpool:
            two_t = cpool.tile([128, B], f32)
            nc.vector.memset(two_t[:], 2.0)
            gamma_t = cpool.tile([128, 1], f32)
            nc.vector.memset(gamma_t[:], float(gamma))
            if not norm_linear:
                sm1_t = cpool.tile([128, B], f32)
                nc.vector.memset(sm1_t[:], float(sm1))

            HB = B // 2  # process 1500-col halves so compute starts earlier
            with tc.tile_pool(name="p1", bufs=3) as p1:
                for t in range(SS // 128):
                    rows = ts(t, 128)
                    code = p1.tile([128, CP], f32, tag="code")
                    normt = p1.tile([128, B], f32, tag="normt")
                    for h in range(2):
                        cols = ts(h, HB)
                        xs = p1.tile([128, HB], f32, tag="xs")
                        nc.sync.dma_start(xs[:], x_in[rows, cols])
                        g1t = p1.tile([128, HB], f32, tag="g1")
                        nc.sync.dma_start(g1t[:], g1_in[rows, cols])
                        g2t = p1.tile([128, HB], f32, tag="g2")
                        nc.sync.dma_start(g2t[:], g2_in[rows, cols])

                        if a_zero:
                            nc.vector.scalar_tensor_tensor(
                                g1t[:], xs[:], -B1, g1t[:], op0=Alu.mult, op1=Alu.add)
                            nc.vector.scalar_tensor_tensor(
                                g2t[:], xs[:], -B2, g2t[:], op0=Alu.mult, op1=Alu.add)
                        else:
                            y = p1.tile([128, HB], f32, tag="y")
                            nc.scalar.activation(y[:], xs[:], Act.Square)
                            nc.vector.scalar_tensor_tensor(
                                g1t[:], y[:], -A1, g1t[:], op0=Alu.mult, op1=Alu.add)
                            nc.vector.scalar_tensor_tensor(
                                g1t[:], xs[:], -B1, g1t[:], op0=Alu.mult, op1=Alu.add)
                            nc.vector.scalar_tensor_tensor(
                                g2t[:], y[:], -A2, g2t[:], op0=Alu.mult, op1=Alu.add)
                            nc.vector.scalar_tensor_tensor(
                                g2t[:], xs[:], -B2, g2t[:], op0=Alu.mult, op1=Alu.add)

                        t01 = code[:, cols]
                        nc.vector.tensor_scalar(t01, g1t[:], 0.0, None, op0=Alu.is_gt)
                        mA = p1.tile([128, HB], f32, tag="mA")
                        nc.scalar.activation(mA[:], g1t[:], Act.Relu)
                        t2 = g2t[:].bitcast(dt.uint32)
                        nc.vector.tensor_tensor(t2, g2t[:], mA[:], op=Alu.is_gt)
                        nc.vector.copy_predicated(t01, t2, two_t[:, 0:HB])
                        if norm_linear:
                            nc.scalar.activation(
                                normt[:, cols], t01, Act.Identity,
                                bias=gamma_t[:, 0:1], scale=float(beta))
                        else:
                            nc.vector.tensor_scalar(
                                normt[:, cols], t01, -sm0, sm0,
                                op0=Alu.mult, op1=Alu.add)
                            nc.vector.copy_predicated(
                                normt[:, cols], t2, sm1_t[:, 0:HB])
                        # f32 -> int8 cast during store (SWDGE), per half so
                        # the first store overlaps the second half's compute;
                        # pad cols garbage, phase 2 compares only [0:B]
                        nc.gpsimd.dma_start(codes_out[rows, cols], code[:, cols])
                        # norm stores go out the ACT-side HWDGE queue so they
                        # don't queue ahead of the next half's loads on sync
                        nc.scalar.dma_start(norm_out[rows, cols], normt[:, cols])
                    nc.gpsimd.dma_start(codes_out[rows, B:CP], code[:, B:CP])

    nc.compile()
    return nc


P2CHUNKS = (256, 640, 640)  # phase-2 chunks: small first chunk starts the
                            # DVE compare ~10us earlier; later chunks' gather
                            # issues (~9ns/row, serial on Q7) hide under the
                            # running compares. (768,768), (256,512,768) and
                            # (384,640,512) all measured slower.


def _build_phase2():
    """Phase-2-only NEFF: codes for all spots arrive as a replicated input;
    gather row/col code rows per edge, count mismatches."""
    import concourse.bacc as bacc
    import concourse.tile as tile
    from concourse import mybir

    dt = mybir.dt
    f32 = dt.float32
    Alu = mybir.AluOpType

    EC = E // M
    NC2 = len(P2CHUNKS)
    nc = bacc.Bacc("TRN2", target_bir_lowering=False, debug=False, num_devices=M)
    codes_in = nc.dram_tensor("codes_full", [S, CP], dt.int8, kind="ExternalInput").ap()
    ridx_in = nc.dram_tensor("ridx", [128, EC // 16], dt.int16, kind="ExternalInput").ap()
    cidx_in = nc.dram_tensor("cidx", [128, EC // 16], dt.int16, kind="ExternalInput").ap()
    cnt_out = nc.dram_tensor("cnt_out", [128, NC2], f32, kind="ExternalOutput").ap()

    with tile.TileContext(nc) as tc:
        with tc.tile_pool(name="c2", bufs=1) as cpool:
            ridx = cpool.tile([128, EC // 16], dt.int16)
            nc.sync.dma_start(ridx[:], ridx_in[:])
            cidx = cpool.tile([128, EC // 16], dt.int16)
            nc.sync.dma_start(cidx[:], cidx_in[:])
            accs = cpool.tile([128, NC2], f32, name="accs")
            nc.vector.memset(accs[:, :], 0.0)

            with tc.tile_pool(name="p2", bufs=1) as p2:
                coff = [sum(P2CHUNKS[:c]) for c in range(NC2)]
                pairs = []
                for ch in range(NC2):
                    che = P2CHUNKS[ch]
                    cols = slice(coff[ch] // 16, (coff[ch] + che) // 16)
                    rt = p2.tile([128, che // 128, CP], dt.int8, tag=f"rt{ch}")
                    nc.gpsimd.dma_gather(
                        rt[:], codes_in[:, :], ridx[:, cols],
                        num_idxs=che, num_idxs_reg=che, elem_size=CP)
                    ct = p2.tile([128, che // 128, CP], dt.int8, tag=f"ct{ch}")
                    nc.gpsimd.dma_gather(
                        ct[:], codes_in[:, :], cidx[:, cols],
                        num_idxs=che, num_idxs_reg=che, elem_size=CP)
                    pairs.append((rt, ct))
                for ch, (rt, ct) in enumerate(pairs):
                    che = P2CHUNKS[ch]
                    scr = p2.tile([128, che // 128, CP], dt.int8, tag="scr")
                    nc.vector.scalar_tensor_tensor(
                        scr[:, :, 0:B],
                        rt[:, :, 0:B],
                        0.0,
                        ct[:, :, 0:B],
                        op0=Alu.bypass,
                        op1=Alu.not_equal,
                        accum_out=accs[:, ch:ch + 1],
                    )
                nc.sync.dma_start(cnt_out[:, :], accs[:, :])

    nc.compile()
    return nc


def _host_prep(x, bin_idx, edge_index, gumbel_noise, state_means, log_stds):
    """Shard + precompute per-core input maps and the baked-scalar config."""
    x = np.asarray(x, dtype=np.float32)
    bin_idx = np.asarray(bin_idx)
    edge_index = np.asarray(edge_index)
    gn = np.asarray(gumbel_noise, dtype=np.float32)
    sm = np.asarray(state_means, dtype=np.float32)
    ls = np.asarray(log_stds, dtype=np.float32)

    if not np.array_equal(bin_idx, np.arange(x.shape[1], dtype=bin_idx.dtype)):
        x = np.ascontiguousarray(x[:, bin_idx])
        gn = np.ascontiguousarray(gn[:, bin_idx, :])

    stds = (np.exp(ls.astype(np.float64)) + 1e-6)
    means = np.array([float(sm[0]), 0.0, float(sm[1])], dtype=np.float64)
    inv_var = 1.0 / (stds * stds)
    # D_k = -(A_k x^2 + B_k x) + (g_k - g_0) - (C_k + log(std_k/std_0))
    A = 0.5 * (inv_var - inv_var[0])
    Bc_ = -(means * inv_var - means[0] * inv_var[0])
    Cc = 0.5 * (means * means * inv_var - means[0] * means[0] * inv_var[0]) \
        + np.log(stds) - np.log(stds[0])

    A1, A2 = np.float32(A[1]), np.float32(A[2])
    B1, B2 = np.float32(Bc_[1]), np.float32(Bc_[2])
    C1, C2 = np.float32(Cc[1]), np.float32(Cc[2])
    a_zero = (A1 == 0.0) and (A2 == 0.0)

    # norm_copy = alpha*code^2 + beta*code + gamma; linear iff alpha == 0
    alpha = (means[0] + means[2]) / 2.0
    beta = -(3.0 * means[0] + means[2]) / 2.0
    gamma = means[0]
    norm_linear = np.float32(alpha) == 0.0

    cfg = (bool(a_zero), bool(norm_linear), float(A1), float(A2),
           float(B1), float(B2), float(np.float32(beta)),
           float(np.float32(gamma)), float(means[0]), float(means[2]))

    G1 = (gn[:, :, 1] - gn[:, :, 0]) - C1
    G2 = (gn[:, :, 2] - gn[:, :, 0]) - C2

    def wrap_idx(idx):
        # element i -> [i % 16, i // 16], replicated 8x down the 128
        # partitions (each GPSIMD Q7 core reads its own 16-partition copy)
        n = idx.shape[0]
        w = np.zeros((16, n // 16), dtype=np.int16)
        w[np.arange(n) % 16, np.arange(n) // 16] = idx.astype(np.int16)
        return np.ascontiguousarray(np.tile(w, (8, 1)))

    # partition edges by row owner; row gathers then read the local shard
    # (overlapping the AllGather), col gathers read the allgathered rows
    # (single AllGather concatenates rank shards in order: row == spot id)
    erow = edge_index[0].astype(np.int64)
    ecol = edge_index[1].astype(np.int64)
    owner = erow // SS

    EC = E // M
    in_maps = []      # single-launch (mode "one"): by-owner edges, padded
    p2_maps = []      # two-launch (mode "two"): plain E/M split, global rows
    for i in range(M):
        r0, r1 = SS * i, SS * (i + 1)
        sel = owner == i
        er = erow[sel] - r0          # local row index into codes_shard
        ec = ecol[sel]               # global row index into codes_full
        n = er.shape[0]
        entry = {
            "x_sh": np.ascontiguousarray(x[r0:r1]),
            "g1_sh": np.ascontiguousarray(G1[r0:r1]),
            "g2_sh": np.ascontiguousarray(G2[r0:r1]),
        }
        if n <= ECAP:
            # pad with self-comparing slots: local row 0 vs its global id
            er_p = np.full(ECAP, 0, dtype=np.int64)
            ec_p = np.full(ECAP, r0, dtype=np.int64)
            er_p[:n] = er
            ec_p[:n] = ec
            entry["ridx"] = wrap_idx(er_p)
            entry["cidx"] = wrap_idx(ec_p)
        in_maps.append(entry)
        p2_maps.append({
            "ridx": wrap_idx(erow[EC * i:EC * (i + 1)]),
            "cidx": wrap_idx(ecol[EC * i:EC * (i + 1)]),
        })
    return in_maps, p2_maps, cfg


def kernel(x, bin_idx, edge_index, gumbel_noise, state_means, log_stds):
    from concourse.bass_utils import run_bass_kernel_spmd

    in_maps, p2_maps, cfg = _host_prep(
        x, bin_idx, edge_index, gumbel_noise, state_means, log_stds)

    global _last_result, _last_results
    _last_results = []
    cores = list(range(M))

    if MODE == "one":
        key = ("one", cfg)
        if key not in _prog_cache:
            _prog_cache[key] = _build_program(cfg)
        res = run_bass_kernel_spmd(_prog_cache[key], in_maps, core_ids=cores,
                                   trace=_TRACE)
        _last_result = res
        _last_results = [res]
        outs = res.results
    else:
        key1 = ("p1", cfg)
        if key1 not in _prog_cache:
            _prog_cache[key1] = _build_phase1(cfg)
        if "p2" not in _prog_cache:
            _prog_cache["p2"] = _build_phase2()
        maps1 = [{k: m[k] for k in ("x_sh", "g1_sh", "g2_sh")} for m in in_maps]
        res1 = run_bass_kernel_spmd(_prog_cache[key1], maps1, core_ids=cores,
                                    trace=_TRACE)
        codes_full = np.concatenate(
            [res1.results[i]["codes_out"] for i in range(M)], axis=0)
        maps2 = [{"codes_full": codes_full, **p2_maps[i]} for i in range(M)]
        res2 = run_bass_kernel_spmd(_prog_cache["p2"], maps2, core_ids=cores,
                                    trace=_TRACE)
        _last_result = res2
        _last_results = [res1, res2]
        outs = [{**res1.results[i], **res2.results[i]} for i in range(M)]

    norm_copy = np.concatenate([outs[i]["norm_out"] for i in range(M)], axis=0)
    total = np.float64(0.0)
    for i in range(M):
        total += np.float64(outs[i]["cnt_out"].sum(dtype=np.float64))
    loss = np.float32(LAMBDA_SMOOTH * 2.0 * total / (E * B * 3))
    return norm_copy.astype(np.float32), loss


# revision 50
# speedup vs baseline: 1.0861x; 1.0254x over previous
"""Trainium2 Bass kernel for nn_DifferentiableHMM_Centered.

Contract: kernel(**inputs) takes FULL unsharded inputs (same keys as
reference.setup_inputs()) and returns the FULL output (norm_copy [S,B] f32,
smooth_loss scalar f32), distributing across 8 NeuronCores internally.

Math reduction
--------------
The reference's straight-through output is, up to ~1ulp wiggle,
  norm_copy[s,b] = all_means[k*(s,b)],
  smooth_loss    = 0.1 * 2 * #{(e,b): k*(row_e,b) != k*(col_e,b)} / (E*Bc*3)
where k*(s,b) = argmax_k [ -0.5 z_k^2 - log(std_k) + gumbel[s,b,k] ],
z_k = (x[s,b]-mean_k)/std_k  (softmax/TAU/one-hot are monotone decorations).

Relative to state 0 the decision statistic is quadratic in x:
  D_k := d_k - d_0 = -(A_k*x^2 + B_k*x) + (g_k - g_0) - const_k
so the device only needs x and the host-precomputed G_k = (g_k-g_0) - const_k.
k* = 0 if D_1<=0 and D_2<=0; 1 if D_1>0 and D_2<=D_1; else 2
(strict comparisons reproduce jnp.argmax first-index tie-breaking).

Device plan (8 cores, SPMD, one NEFF):
  phase 1 (spots sharded 256/core): D_k via fused scalar_tensor_tensor on DVE
    (all per-state scalars baked as immediates), masks t01/t2, codes {0,1,2}
    + norm_copy (ACT Identity affine of the code); codes cast f32->int8 on
    store. All elementwise runs on DVE/ACT -- GpSimd compute contends with
    DVE on SBUF ports (measured 15x slowdown), so GpSimd only drives DMA.
  AllGather of int8 codes (rows padded to 3072B), split per 128-spot tile so
    the first AG overlaps the second tile's compute.
  phase 2 (edges sharded 1536/core): dma_gather of row/col code rows,
    fused scalar_tensor_tensor(bypass, not_equal, accum_out) -> counts.
Host sums the 8x128xNCH partial counts into smooth_loss.
"""

import numpy as np

S, B, E = 2048, 3000, 12288
M = 8            # cores
SS = S // M      # 256 spots per core
CP = 3072        # padded code row length (int8 bytes), multiple of 256
CHUNKS = (896, 768)  # phase-2 chunk sizes (multiples of 128)
ECAP = sum(CHUNKS)   # per-core edge capacity (edges partitioned by row owner,
                     # padded with self-comparing slots; E/M=1536, max seen 1576)
NCH = len(CHUNKS)
TAU = 0.1
LAMBDA_SMOOTH = 0.1

_prog_cache: dict = {}
_TRACE = False       # set by test harness for profiling runs
_last_result = None  # BassKernelResults of the most recent launch
_last_results = []   # all launches of the most recent kernel() call
MODE = "two"         # "two" = phase1 NEFF + host relay + phase2 NEFF
                     # "one" = single NEFF with on-device AllGather


def _build_program(cfg, do_p1=True, do_ag=True, do_p2=True, split_ag=True):
    """cfg: (a_zero, norm_linear, A1, A2, B1, B2, beta, gamma, sm0, sm1).
    All scalars are baked into the BIR as immediates."""
    import concourse.bass as bass
    import concourse.bacc as bacc
    import concourse.tile as tile
    from concourse import mybir

    a_zero, norm_linear, A1, A2, B1, B2, beta, gamma, sm0, sm1 = cfg

    dt = mybir.dt
    f32 = dt.float32
    Alu = mybir.AluOpType
    Act = mybir.ActivationFunctionType
    ts = bass.ts

    nc = bacc.Bacc(
        "TRN2",
        target_bir_lowering=False,
        debug=False,
        num_devices=M,
    )

    x_in = nc.dram_tensor("x_sh", [SS, B], f32, kind="ExternalInput").ap()
    g1_in = nc.dram_tensor("g1_sh", [SS, B], f32, kind="ExternalInput").ap()
    g2_in = nc.dram_tensor("g2_sh", [SS, B], f32, kind="ExternalInput").ap()
    ridx_in = nc.dram_tensor("ridx", [128, ECAP // 16], dt.int16, kind="ExternalInput").ap()
    cidx_in = nc.dram_tensor("cidx", [128, ECAP // 16], dt.int16, kind="ExternalInput").ap()
    norm_out = nc.dram_tensor("norm_out", [SS, B], f32, kind="ExternalOutput").ap()
    cnt_out = nc.dram_tensor("cnt_out", [128, NCH], f32, kind="ExternalOutput").ap()

    NT = SS // 128  # spot tiles per core (2)

    with tile.TileContext(nc) as tc:
        with tc.tile_pool(name="dram", bufs=1, space="DRAM") as dpool, \
             tc.tile_pool(name="consts", bufs=1) as cpool:

            codes_shard = dpool.tile([SS, CP], dt.int8)
            codes_full = dpool.tile([S, CP], dt.int8, addr_space="Shared")

            # constant data tile for copy_predicated (code 2 where state-2 wins)
            two_t = cpool.tile([128, B], f32)
            nc.vector.memset(two_t[:], 2.0)
            gamma_t = cpool.tile([128, 1], f32)
            nc.vector.memset(gamma_t[:], float(gamma))
            if not norm_linear:
                sm1_t = cpool.tile([128, B], f32)
                nc.vector.memset(sm1_t[:], float(sm1))

            ridx = cpool.tile([128, ECAP // 16], dt.int16)
            nc.sync.dma_start(ridx[:], ridx_in[:])
            cidx = cpool.tile([128, ECAP // 16], dt.int16)
            nc.sync.dma_start(cidx[:], cidx_in[:])

            with tc.tile_pool(name="p1", bufs=2) as p1:
                for t in range(NT if do_p1 else 0):
                    rows = ts(t, 128)
                    xs = p1.tile([128, B], f32, tag="xs")
                    nc.sync.dma_start(xs[:], x_in[rows, :])
                    g1t = p1.tile([128, B], f32, tag="g1")
                    nc.sync.dma_start(g1t[:], g1_in[rows, :])
                    g2t = p1.tile([128, B], f32, tag="g2")
                    nc.sync.dma_start(g2t[:], g2_in[rows, :])

                    # D_k computed in place over G_k tiles (DVE)
                    if a_zero:
                        nc.vector.scalar_tensor_tensor(
                            g1t[:], xs[:], -B1, g1t[:],
                            op0=Alu.mult, op1=Alu.add)
                        nc.vector.scalar_tensor_tensor(
                            g2t[:], xs[:], -B2, g2t[:],
                            op0=Alu.mult, op1=Alu.add)
                    else:
                        y = p1.tile([128, B], f32, tag="y")
                        nc.scalar.activation(y[:], xs[:], Act.Square)
                        nc.vector.scalar_tensor_tensor(
                            g1t[:], y[:], -A1, g1t[:],
                            op0=Alu.mult, op1=Alu.add)
                        nc.vector.scalar_tensor_tensor(
                            g1t[:], xs[:], -B1, g1t[:],
                            op0=Alu.mult, op1=Alu.add)
                        nc.vector.scalar_tensor_tensor(
                            g2t[:], y[:], -A2, g2t[:],
                            op0=Alu.mult, op1=Alu.add)
                        nc.vector.scalar_tensor_tensor(
                            g2t[:], xs[:], -B2, g2t[:],
                            op0=Alu.mult, op1=Alu.add)

                    code = p1.tile([128, CP], f32, tag="code")
                    t01 = code[:, 0:B]
                    # t01 = [D1 > 0]  (DVE tensor_scalar, 2x mode)
                    nc.vector.tensor_scalar(t01, g1t[:], 0.0, None, op0=Alu.is_gt)
                    # mA = max(D1, 0) = relu(D1)  (ACT)
                    mA = p1.tile([128, B], f32, tag="mA")
                    nc.scalar.activation(mA[:], g1t[:], Act.Relu)
                    # t2 = [D2 > mA], in place over D2, uint32 mask for
                    # copy_predicated (integer mask required)
                    t2 = g2t[:].bitcast(dt.uint32)
                    nc.vector.tensor_tensor(t2, g2t[:], mA[:], op=Alu.is_gt)
                    nc.vector.copy_predicated(t01, t2, two_t[:])
                    # pad cols [B:CP] stay uninitialized -- phase 2 compares
                    # only [:, :, 0:B] of each gathered row

                    normt = p1.tile([128, B], f32, tag="normt")
                    if norm_linear:
                        # norm = beta*code + gamma  (ACT Identity affine)
                        nc.scalar.activation(
                            normt[:], t01, Act.Identity,
                            bias=gamma_t[:, 0:1], scale=float(beta))
                    else:
                        nc.vector.tensor_scalar(
                            normt[:], t01, -sm0, sm0, op0=Alu.mult, op1=Alu.add)
                        nc.vector.copy_predicated(normt[:], t2, sm1_t[:])
                    nc.sync.dma_start(norm_out[rows, :], normt[:])
                    # f32 -> int8 cast during store (SWDGE)
                    nc.gpsimd.dma_start(codes_shard[rows, :], code[:])

            if do_ag:
                nc.gpsimd.collective_compute(
                    "AllGather",
                    Alu.bypass,
                    replica_groups=[list(range(M))],
                    ins=[codes_shard[:, :].opt()],
                    outs=[codes_full[:, :].opt()],
                )

            with tc.tile_pool(name="p2", bufs=1) as p2:
                accs = cpool.tile([128, NCH], f32, name="accs")
                nc.vector.memset(accs[:, :], 0.0)
                coff = [sum(CHUNKS[:c]) for c in range(NCH)]
                gathered = []
                # row gathers read the LOCAL codes_shard (edges are
                # partitioned by row owner) -> they overlap the AllGather
                for ch in range(NCH if do_p2 else 0):
                    che = CHUNKS[ch]
                    cols = slice(coff[ch] // 16, (coff[ch] + che) // 16)
                    rt = p2.tile([128, che // 128, CP], dt.int8, tag=f"rt{ch}")
                    nc.gpsimd.dma_gather(
                        rt[:], codes_shard[:, :], ridx[:, cols],
                        num_idxs=che, num_idxs_reg=che, elem_size=CP)
                    gathered.append(rt)
                for ch in range(NCH if do_p2 else 0):
                    che = CHUNKS[ch]
                    cols = slice(coff[ch] // 16, (coff[ch] + che) // 16)
                    ct = p2.tile([128, che // 128, CP], dt.int8, tag=f"ct{ch}")
                    nc.gpsimd.dma_gather(
                        ct[:], codes_full[:, :], cidx[:, cols],
                        num_idxs=che, num_idxs_reg=che, elem_size=CP)
                    rt = gathered[ch]
                    scr = p2.tile([128, che // 128, CP], dt.int8, tag="scr")
                    # compare only the 3000 real code bytes of each row
                    nc.vector.scalar_tensor_tensor(
                        scr[:, :, 0:B],
                        rt[:, :, 0:B],
                        0.0,
                        ct[:, :, 0:B],
                        op0=Alu.bypass,
                        op1=Alu.not_equal,
                        accum_out=accs[:, ch:ch + 1],
                    )
                nc.sync.dma_start(cnt_out[:, :], accs[:, :])

    nc.compile()
    return nc


def _build_phase1(cfg):
    """Phase-1-only NEFF: no collectives -> no NRT comm-init barrier.
    Outputs norm_copy shard + int8 codes shard (padded rows)."""
    import concourse.bacc as bacc
    import concourse.tile as tile
    from concourse import mybir

    a_zero, norm_linear, A1, A2, B1, B2, beta, gamma, sm0, sm1 = cfg
    dt = mybir.dt
    f32 = dt.float32
    Alu = mybir.AluOpType
    Act = mybir.ActivationFunctionType
    import concourse.bass as bass
    ts = bass.ts

    nc = bacc.Bacc("TRN2", target_bir_lowering=False, debug=False, num_devices=M)
    x_in = nc.dram_tensor("x_sh", [SS, B], f32, kind="ExternalInput").ap()
    g1_in = nc.dram_tensor("g1_sh", [SS, B], f32, kind="ExternalInput").ap()
    g2_in = nc.dram_tensor("g2_sh", [SS, B], f32, kind="ExternalInput").ap()
    norm_out = nc.dram_tensor("norm_out", [SS, B], f32, kind="ExternalOutput").ap()
    codes_out = nc.dram_tensor("codes_out", [SS, CP], dt.int8, kind="ExternalOutput").ap()

    with tile.TileContext(nc) as tc:
        with tc.tile_pool(name="consts", bufs=1) as cpool:
            two_t = cpool.tile([128, B], f32)
            nc.vector.memset(two_t[:], 2.0)
            gamma_t = cpool.tile([128, 1], f32)
            nc.vector.memset(gamma_t[:], float(gamma))
            if not norm_linear:
                sm1_t = cpool.tile([128, B], f32)
                nc.vector.memset(sm1_t[:], float(sm1))

            HB = B // 2  # process 1500-col halves so compute starts earlier
            with tc.tile_pool(name="p1", bufs=3) as p1:
                for t in range(SS // 128):
                    rows = ts(t, 128)
                    code = p1.tile([128, CP], f32, tag="code")
                    normt = p1.tile([128, B], f32, tag="normt")
                    for h in range(2):
                        cols = ts(h, HB)
                        xs = p1.tile([128, HB], f32, tag="xs")
                        nc.sync.dma_start(xs[:], x_in[rows, cols])
                        g1t = p1.tile([128, HB], f32, tag="g1")
                        nc.sync.dma_start(g1t[:], g1_in[rows, cols])
                        g2t = p1.tile([128, HB], f32, tag="g2")
                        nc.sync.dma_start(g2t[:], g2_in[rows, cols])

                        if a_zero:
                            nc.vector.scalar_tensor_tensor(
                                g1t[:], xs[:], -B1, g1t[:], op0=Alu.mult, op1=Alu.add)
                            nc.vector.scalar_tensor_tensor(
                                g2t[:], xs[:], -B2, g2t[:], op0=Alu.mult, op1=Alu.add)
                        else:
                            y = p1.tile([128, HB], f32, tag="y")
                            nc.scalar.activation(y[:], xs[:], Act.Square)
                            nc.vector.scalar_tensor_tensor(
                                g1t[:], y[:], -A1, g1t[:], op0=Alu.mult, op1=Alu.add)
                            nc.vector.scalar_tensor_tensor(
                                g1t[:], xs[:], -B1, g1t[:], op0=Alu.mult, op1=Alu.add)
                            nc.vector.scalar_tensor_tensor(
                                g2t[:], y[:], -A2, g2t[:], op0=Alu.mult, op1=Alu.add)
                            nc.vector.scalar_tensor_tensor(
                                g2t[:], xs[:], -B2, g2t[:], op0=Alu.mult, op1=Alu.add)

                        t01 = code[:, cols]
                        nc.vector.tensor_scalar(t01, g1t[:], 0.0, None, op0=Alu.is_gt)
                        mA = p1.tile([128, HB], f32, tag="mA")
                        nc.scalar.activation(mA[:], g1t[:], Act.Relu)
                        t2 = g2t[:].bitcast(dt.uint32)
                        nc.vector.tensor_tensor(t2, g2t[:], mA[:], op=Alu.is_gt)
                        nc.vector.copy_predicated(t01, t2, two_t[:, 0:HB])
                        if norm_linear:
                            nc.scalar.activation(
                                normt[:, cols], t01, Act.Identity,
                                bias=gamma_t[:, 0:1], scale=float(beta))
                        else:
                            nc.vector.tensor_scalar(
                                normt[:, cols], t01, -sm0, sm0,
                                op0=Alu.mult, op1=Alu.add)
                            nc.vector.copy_predicated(
                                normt[:, cols], t2, sm1_t[:, 0:HB])
                        # f32 -> int8 cast during store (SWDGE), per half so
                        # the first store overlaps the second half's compute;
                        # pad cols garbage, phase 2 compares only [0:B]
                        nc.gpsimd.dma_start(codes_out[rows, cols], code[:, cols])
                        # norm stores go out the ACT-side HWDGE queue so they
                        # don't queue ahead of the next half's loads on sync
                        nc.scalar.dma_start(norm_out[rows, cols], normt[:, cols])
                    nc.gpsimd.dma_start(codes_out[rows, B:CP], code[:, B:CP])

    nc.compile()
    return nc


P2CHUNKS = (256, 640, 640)  # phase-2 chunks: small first chunk starts the
                            # DVE compare ~10us earlier; later chunks' gather
                            # issues (~9ns/row, serial on Q7) hide under the
                            # running compares. (768,768), (256,512,768) and
                            # (384,640,512) all measured slower.


def _build_phase2():
    """Phase-2-only NEFF: codes for all spots arrive as a replicated input;
    gather row/col code rows per edge, count mismatches."""
    import concourse.bacc as bacc
    import concourse.tile as tile
    from concourse import mybir

    dt = mybir.dt
    f32 = dt.float32
    Alu = mybir.AluOpType

    EC = E // M
    NC2 = len(P2CHUNKS)
    nc = bacc.Bacc("TRN2", target_bir_lowering=False, debug=False, num_devices=M)
    codes_in = nc.dram_tensor("codes_full", [S, CP], dt.int8, kind="ExternalInput").ap()
    ridx_in = nc.dram_tensor("ridx", [128, EC // 16], dt.int16, kind="ExternalInput").ap()
    cidx_in = nc.dram_tensor("cidx", [128, EC // 16], dt.int16, kind="ExternalInput").ap()
    cnt_out = nc.dram_tensor("cnt_out", [128, NC2], f32, kind="ExternalOutput").ap()

    with tile.TileContext(nc) as tc:
        with tc.tile_pool(name="c2", bufs=1) as cpool:
            ridx = cpool.tile([128, EC // 16], dt.int16)
            nc.sync.dma_start(ridx[:], ridx_in[:])
            cidx = cpool.tile([128, EC // 16], dt.int16)
            nc.sync.dma_start(cidx[:], cidx_in[:])
            accs = cpool.tile([128, NC2], f32, name="accs")
            nc.vector.memset(accs[:, :], 0.0)

            with tc.tile_pool(name="p2", bufs=1) as p2:
                coff = [sum(P2CHUNKS[:c]) for c in range(NC2)]
                pairs = []
                for ch in range(NC2):
                    che = P2CHUNKS[ch]
                    cols = slice(coff[ch] // 16, (coff[ch] + che) // 16)
                    rt = p2.tile([128, che // 128, CP], dt.int8, tag=f"rt{ch}")
                    nc.gpsimd.dma_gather(
                        rt[:], codes_in[:, :], ridx[:, cols],
                        num_idxs=che, num_idxs_reg=che, elem_size=CP,
                        single_packet=False)
                    ct = p2.tile([128, che // 128, CP], dt.int8, tag=f"ct{ch}")
                    nc.gpsimd.dma_gather(
                        ct[:], codes_in[:, :], cidx[:, cols],
                        num_idxs=che, num_idxs_reg=che, elem_size=CP,
                        single_packet=False)
                    pairs.append((rt, ct))
                for ch, (rt, ct) in enumerate(pairs):
                    che = P2CHUNKS[ch]
                    scr = p2.tile([128, che // 128, CP], dt.int8, tag="scr")
                    nc.vector.scalar_tensor_tensor(
                        scr[:, :, 0:B],
                        rt[:, :, 0:B],
                        0.0,
                        ct[:, :, 0:B],
                        op0=Alu.bypass,
                        op1=Alu.not_equal,
                        accum_out=accs[:, ch:ch + 1],
                    )
                nc.sync.dma_start(cnt_out[:, :], accs[:, :])

    nc.compile()
    return nc


def _host_prep(x, bin_idx, edge_index, gumbel_noise, state_means, log_stds):
    """Shard + precompute per-core input maps and the baked-scalar config."""
    x = np.asarray(x, dtype=np.float32)
    bin_idx = np.asarray(bin_idx)
    edge_index = np.asarray(edge_index)
    gn = np.asarray(gumbel_noise, dtype=np.float32)
    sm = np.asarray(state_means, dtype=np.float32)
    ls = np.asarray(log_stds, dtype=np.float32)

    if not np.array_equal(bin_idx, np.arange(x.shape[1], dtype=bin_idx.dtype)):
        x = np.ascontiguousarray(x[:, bin_idx])
        gn = np.ascontiguousarray(gn[:, bin_idx, :])

    stds = (np.exp(ls.astype(np.float64)) + 1e-6)
    means = np.array([float(sm[0]), 0.0, float(sm[1])], dtype=np.float64)
    inv_var = 1.0 / (stds * stds)
    # D_k = -(A_k x^2 + B_k x) + (g_k - g_0) - (C_k + log(std_k/std_0))
    A = 0.5 * (inv_var - inv_var[0])
    Bc_ = -(means * inv_var - means[0] * inv_var[0])
    Cc = 0.5 * (means * means * inv_var - means[0] * means[0] * inv_var[0]) \
        + np.log(stds) - np.log(stds[0])

    A1, A2 = np.float32(A[1]), np.float32(A[2])
    B1, B2 = np.float32(Bc_[1]), np.float32(Bc_[2])
    C1, C2 = np.float32(Cc[1]), np.float32(Cc[2])
    a_zero = (A1 == 0.0) and (A2 == 0.0)

    # norm_copy = alpha*code^2 + beta*code + gamma; linear iff alpha == 0
    alpha = (means[0] + means[2]) / 2.0
    beta = -(3.0 * means[0] + means[2]) / 2.0
    gamma = means[0]
    norm_linear = np.float32(alpha) == 0.0

    cfg = (bool(a_zero), bool(norm_linear), float(A1), float(A2),
           float(B1), float(B2), float(np.float32(beta)),
           float(np.float32(gamma)), float(means[0]), float(means[2]))

    G1 = (gn[:, :, 1] - gn[:, :, 0]) - C1
    G2 = (gn[:, :, 2] - gn[:, :, 0]) - C2

    def wrap_idx(idx):
        # element i -> [i % 16, i // 16], replicated 8x down the 128
        # partitions (each GPSIMD Q7 core reads its own 16-partition copy)
        n = idx.shape[0]
        w = np.zeros((16, n // 16), dtype=np.int16)
        w[np.arange(n) % 16, np.arange(n) // 16] = idx.astype(np.int16)
        return np.ascontiguousarray(np.tile(w, (8, 1)))

    # partition edges by row owner; row gathers then read the local shard
    # (overlapping the AllGather), col gathers read the allgathered rows
    # (single AllGather concatenates rank shards in order: row == spot id)
    erow = edge_index[0].astype(np.int64)
    ecol = edge_index[1].astype(np.int64)
    owner = erow // SS

    EC = E // M
    in_maps = []      # single-launch (mode "one"): by-owner edges, padded
    p2_maps = []      # two-launch (mode "two"): plain E/M split, global rows
    for i in range(M):
        r0, r1 = SS * i, SS * (i + 1)
        sel = owner == i
        er = erow[sel] - r0          # local row index into codes_shard
        ec = ecol[sel]               # global row index into codes_full
        n = er.shape[0]
        entry = {
            "x_sh": np.ascontiguousarray(x[r0:r1]),
            "g1_sh": np.ascontiguousarray(G1[r0:r1]),
            "g2_sh": np.ascontiguousarray(G2[r0:r1]),
        }
        if n <= ECAP:
            # pad with self-comparing slots: local row 0 vs its global id
            er_p = np.full(ECAP, 0, dtype=np.int64)
            ec_p = np.full(ECAP, r0, dtype=np.int64)
            er_p[:n] = er
            ec_p[:n] = ec
            entry["ridx"] = wrap_idx(er_p)
            entry["cidx"] = wrap_idx(ec_p)
        in_maps.append(entry)
        p2_maps.append({
            "ridx": wrap_idx(erow[EC * i:EC * (i + 1)]),
            "cidx": wrap_idx(ecol[EC * i:EC * (i + 1)]),
        })
    return in_maps, p2_maps, cfg


def kernel(x, bin_idx, edge_index, gumbel_noise, state_means, log_stds):
    from concourse.bass_utils import run_bass_kernel_spmd

    in_maps, p2_maps, cfg = _host_prep(
        x, bin_idx, edge_index, gumbel_noise, state_means, log_stds)

    global _last_result, _last_results
    _last_results = []
    cores = list(range(M))

    if MODE == "one":
        key = ("one", cfg)
        if key not in _prog_cache:
            _prog_cache[key] = _build_program(cfg)
        res = run_bass_kernel_spmd(_prog_cache[key], in_maps, core_ids=cores,
                                   trace=_TRACE)
        _last_result = res
        _last_results = [res]
        outs = res.results
    else:
        key1 = ("p1", cfg)
        if key1 not in _prog_cache:
            _prog_cache[key1] = _build_phase1(cfg)
        if "p2" not in _prog_cache:
            _prog_cache["p2"] = _build_phase2()
        maps1 = [{k: m[k] for k in ("x_sh", "g1_sh", "g2_sh")} for m in in_maps]
        res1 = run_bass_kernel_spmd(_prog_cache[key1], maps1, core_ids=cores,
                                    trace=_TRACE)
        codes_full = np.concatenate(
            [res1.results[i]["codes_out"] for i in range(M)], axis=0)
        maps2 = [{"codes_full": codes_full, **p2_maps[i]} for i in range(M)]
        res2 = run_bass_kernel_spmd(_prog_cache["p2"], maps2, core_ids=cores,
                                    trace=_TRACE)
        _last_result = res2
        _last_results = [res1, res2]
        outs = [{**res1.results[i], **res2.results[i]} for i in range(M)]

    norm_copy = np.concatenate([outs[i]["norm_out"] for i in range(M)], axis=0)
    total = np.float64(0.0)
    for i in range(M):
        total += np.float64(outs[i]["cnt_out"].sum(dtype=np.float64))
    loss = np.float32(LAMBDA_SMOOTH * 2.0 * total / (E * B * 3))
    return norm_copy.astype(np.float32), loss
